# revision 7
# baseline (speedup 1.0000x reference)
"""TRN2 Bass kernel for nn_DecoderBlock (B=4, T=2048, D=1024, H=16, HID=4096).

Sharding: 8 cores = batch(4) x causal-balanced T-split(2).  Core (b, p) owns
token blocks {256*s + 128*p : +128, s=0..7} of batch b (interleaved 128-row
blocks; per-core causal work is exactly balanced).  Each core redundantly
computes LN1 + K/V over the full sequence of its batch element, so there is
no cross-core communication at all; attention Q/scores/output, proj, LN2 and
the MLP are computed only for the core's own 1024 rows.  Host reassembles the
8 [1024, 1024] outputs.

Token columns are permuted per core to [own | other] so the SPMD program is
identical across cores; the only data-dependent piece of causal masking is a
[128,128] triangular constant and a [128,128] all-0/all-1 flag tile.

Layouts (per core):
  xT     [D, T]    fp32   x[b].T (token-permuted)
  ln1T   [D, T]    bf16   layernorm1, transposed; stats via PE ones-matmul
  QT     [D, TOWN] bf16   Q^T   KT [D, T] bf16   K^T   (weights-stationary)
  V_aug  [128, T/128, H, 65] bf16  V natural + ones column (softmax denom)
  P^T    [128, tail] bf16  exp(scores^T), per k-block, masked
  O      [128, 65] psum    P^T-stationary AV matmul -> per-q denom in col 64
  attn   [TOWN, D] bf16 natural -> PE-transposed for proj
  x2/out fp32 residual stream end-to-end
"""
import numpy as np
import ml_dtypes
from contextlib import ExitStack

import concourse.bass as bass
import concourse.bacc as bacc
import concourse.mybir as mybir
import concourse.tile as tile
from concourse.masks import make_identity

def _chunks(total, step=512):
    return [(i, min(step, total - i)) for i in range(0, total, step)]


F32 = mybir.dt.float32
BF16 = mybir.dt.bfloat16
AF = mybir.ActivationFunctionType
ALU = mybir.AluOpType

FULL_CFG = dict(T=2048, D=1024, HID=4096)
EPS = 1e-5


def build_program(cfg):
    T, D, HID = cfg["T"], cfg["D"], cfg["HID"]
    HD = 64
    p = dict(
        T=T, D=D, HID=HID,
        H=D // HD, DC=D // 128, HC=HID // 128,
        TOWN=T // 2, NS=T // 256, NKB=T // 128, TC=T // 128,
        ISC=float(1.0 / np.sqrt(np.float32(HD))),
    )
    nc = bacc.Bacc("TRN2", target_bir_lowering=False, debug=False)

    def din(name, shape, dtype=F32):
        return nc.dram_tensor(name, shape, dtype, kind="ExternalInput").ap()

    DC, HC = p["DC"], p["HC"]
    t = dict(
        xT=din("xT", [D, T]),
        x_own=din("x_own", [p["TOWN"], D]),
        tri=din("tri", [128, 128], BF16),
        flag=din("flag", [128, 128], BF16),
        wq=din("wq", [D, D], BF16),
        wk=din("wk", [D, D], BF16),
        wv=din("wv", [D, D], BF16),
        qb_pp=din("qb_pp", [128, DC]),
        kb_pp=din("kb_pp", [128, DC]),
        vb_row=din("vb_row", [1, D]),
        ln1g_pp=din("ln1g_pp", [128, DC]),
        ln1b_pp=din("ln1b_pp", [128, DC]),
        ln2g_pp=din("ln2g_pp", [128, DC]),
        ln2b_pp=din("ln2b_pp", [128, DC]),
        wproj=din("wproj", [D, D], BF16),
        projb_row=din("projb_row", [1, D]),
        wfc1t=din("wfc1t", [HC, 128, DC, 128], BF16),
        fc1b_pp=din("fc1b_pp", [128, HC]),
        wfc2=din("wfc2", [HID, D], BF16),
        fc2b_row=din("fc2b_row", [1, D]),
        out=nc.dram_tensor("out", [p["TOWN"], D], F32, kind="ExternalOutput").ap(),
    )
    with tile.TileContext(nc) as tc:
        _body(tc, p, t)
    nc.compile()
    return nc


def _body(tc, p, t):
    nc = tc.nc
    T, D, HID = p["T"], p["D"], p["HID"]
    H, DC, HC = p["H"], p["DC"], p["HC"]
    TOWN, NS, NKB, TC, ISC = p["TOWN"], p["NS"], p["NKB"], p["TC"], p["ISC"]

    with ExitStack() as top:
        consts = top.enter_context(tc.tile_pool(name="consts", bufs=1))
        ident = consts.tile([128, 128], BF16, tag="ident")
        make_identity(nc, ident[:])
        tri_t = consts.tile([128, 128], BF16, tag="tri")
        nc.sync.dma_start(tri_t[:], t["tri"][:])
        flag_t = consts.tile([128, 128], BF16, tag="flag")
        nc.sync.dma_start(flag_t[:], t["flag"][:])
        ppvec = consts.tile([128, DC, 6], F32, tag="ppvec")
        for i, nm in enumerate(["qb_pp", "kb_pp", "ln1g_pp", "ln1b_pp",
                                "ln2g_pp", "ln2b_pp"]):
            nc.sync.dma_start(ppvec[:, :, i], t[nm][:])
        qb_t, kb_t = ppvec[:, :, 0], ppvec[:, :, 1]
        ln1g_t, ln1b_t = ppvec[:, :, 2], ppvec[:, :, 3]
        ln2g_t, ln2b_t = ppvec[:, :, 4], ppvec[:, :, 5]
        fc1b_t = consts.tile([128, HC], F32, tag="fc1b")
        nc.sync.dma_start(fc1b_t[:], t["fc1b_pp"][:])
        vb_b = consts.tile([128, D], F32, tag="vb_b")
        nc.sync.dma_start(vb_b[:], t["vb_row"][:].broadcast_to([128, D]))
        projb_b = consts.tile([128, D], F32, tag="projb_b")
        nc.sync.dma_start(projb_b[:], t["projb_row"][:].broadcast_to([128, D]))
        fc2b_b = consts.tile([128, D], F32, tag="fc2b_b")
        nc.sync.dma_start(fc2b_b[:], t["fc2b_row"][:].broadcast_to([128, D]))
        ones_f = consts.tile([128, 128], F32, tag="ones_f")
        nc.vector.memset(ones_f[:], 1.0)
        eps_t = consts.tile([128, 1], F32, tag="eps")
        nc.vector.memset(eps_t[:], EPS)

        attnp = top.enter_context(tc.tile_pool(name="attnp", bufs=1))
        attn = attnp.tile([128, NS, D], BF16, tag="attn")

        with ExitStack() as qsc:  # LN1 + QKV + attention
            qkvp = qsc.enter_context(tc.tile_pool(name="qkvp", bufs=1))
            QT = qkvp.tile([128, DC, TOWN], BF16, tag="QT")
            KT = qkvp.tile([128, DC, T], BF16, tag="KT")
            Vaug = qkvp.tile([128, TC, H, 65], BF16, tag="Vaug")
            nc.gpsimd.memset(Vaug[:, :, :, 64:65], 1.0)

            with ExitStack() as lsc:  # LN1 + QKV
                lnq = lsc.enter_context(tc.tile_pool(name="lnq", bufs=1))
                ln1T = lnq.tile([128, DC, T], BF16, tag="ln1T")
                with ExitStack() as l1:
                    lnp = l1.enter_context(tc.tile_pool(name="lnp", bufs=2))
                    lns = l1.enter_context(tc.tile_pool(name="lns", bufs=1))
                    lps = l1.enter_context(
                        tc.tile_pool(name="lnps", bufs=1, space="PSUM"))
                    S1b = lps.tile([128, T], F32, tag="S1b")
                    S2b = lps.tile([128, T], F32, tag="S2b")
                    for c in range(DC):
                        xc = lnp.tile([128, T], F32, tag="xc")
                        nc.sync.dma_start(xc[:], t["xT"][c * 128:(c + 1) * 128, :])
                        sq = lnp.tile([128, T], F32, tag="scr")
                        nc.scalar.activation(sq[:], xc[:], AF.Square)
                        for n0, w in _chunks(T):
                            sl = bass.ds(n0, w)
                            nc.tensor.matmul(S1b[:, sl], ones_f[:], xc[:, sl],
                                             start=(c == 0), stop=(c == DC - 1))
                            nc.tensor.matmul(S2b[:, sl], ones_f[:], sq[:, sl],
                                             start=(c == 0), stop=(c == DC - 1))
                    # stats in broadcast layout [128, T]; A=mu, B=ex2->var->rstd, C=scratch->nmr
                    A = lns.tile([128, T], F32, tag="A")
                    B = lns.tile([128, T], F32, tag="B")
                    C = lns.tile([128, T], F32, tag="C")
                    nc.vector.tensor_scalar_mul(A[:], S1b[:], 1.0 / D)
                    nc.vector.tensor_scalar_mul(B[:], S2b[:], 1.0 / D)
                    nc.vector.scalar_tensor_tensor(
                        C[:], A[:], -1.0, A[:], op0=ALU.mult, op1=ALU.mult)
                    nc.vector.tensor_add(B[:], B[:], C[:])          # var
                    nc.scalar.activation(C[:], B[:], AF.Sqrt, bias=eps_t[:])
                    nc.vector.reciprocal(B[:], C[:])                # rstd
                    nc.vector.scalar_tensor_tensor(
                        C[:], A[:], -1.0, B[:], op0=ALU.mult, op1=ALU.mult)  # -mu*rstd
                    for c in range(DC):
                        xc = lnp.tile([128, T], F32, tag="xc")
                        nc.sync.dma_start(xc[:], t["xT"][c * 128:(c + 1) * 128, :])
                        t1 = lnp.tile([128, T], F32, tag="scr")
                        nc.vector.tensor_mul(t1[:], xc[:], B[:])
                        nc.vector.tensor_add(t1[:], t1[:], C[:])
                        nc.scalar.activation(
                            ln1T[:, c, :], t1[:], AF.Identity,
                            scale=ln1g_t[:, c:c + 1], bias=ln1b_t[:, c:c + 1])

                # ---- QKV (weights streamed per D-chunk, psum per out-chunk)
                wst = lsc.enter_context(tc.tile_pool(name="wst", bufs=2))
                qps = lsc.enter_context(
                    tc.tile_pool(name="qps", bufs=2, space="PSUM"))
                w_t = [None] * DC
                for c in range(DC):
                    w_t[c] = wst.tile([128, D], BF16, tag=f"w{c}", name=f"w{c}")
                    nc.sync.dma_start(w_t[c][:], t["wq"][c * 128:(c + 1) * 128, :])
                for qc in range(DC):
                    ps = qps.tile([128, TOWN], F32, tag="qkv_ps")
                    for c in range(DC):
                        for n0, w in _chunks(TOWN):
                            sl = bass.ds(n0, w)
                            nc.tensor.matmul(ps[:, sl],
                                             w_t[c][:, qc * 128:(qc + 1) * 128],
                                             ln1T[:, c, 0:TOWN][:, sl],
                                             start=(c == 0), stop=(c == DC - 1))
                    nc.scalar.activation(QT[:, qc, :], ps[:], AF.Identity,
                                         bias=qb_t[:, qc:qc + 1])
                for c in range(DC):
                    w_t[c] = wst.tile([128, D], BF16, tag=f"w{c}", name=f"w{c}")
                    nc.sync.dma_start(w_t[c][:], t["wk"][c * 128:(c + 1) * 128, :])
                for kc in range(DC):
                    for th in range(T // TOWN):
                        ps = qps.tile([128, TOWN], F32, tag="qkv_ps")
                        base = th * TOWN
                        for c in range(DC):
                            for n0, w in _chunks(TOWN):
                                nc.tensor.matmul(
                                    ps[:, bass.ds(n0, w)],
                                    w_t[c][:, kc * 128:(kc + 1) * 128],
                                    ln1T[:, c, :][:, bass.ds(base + n0, w)],
                                    start=(c == 0), stop=(c == DC - 1))
                        nc.scalar.activation(KT[:, kc, base:base + TOWN], ps[:],
                                             AF.Identity, bias=kb_t[:, kc:kc + 1])
                for c in range(DC):
                    w_t[c] = wst.tile([128, D], BF16, tag=f"w{c}", name=f"w{c}")
                    nc.sync.dma_start(w_t[c][:], t["wv"][c * 128:(c + 1) * 128, :])
                for tcn in range(TC):
                    ps = qps.tile([128, D], F32, tag="qkv_ps")
                    for c in range(DC):
                        for n0, w in _chunks(D):
                            sl = bass.ds(n0, w)
                            nc.tensor.matmul(ps[:, sl],
                                             ln1T[:, c, tcn * 128:(tcn + 1) * 128],
                                             w_t[c][:, sl],
                                             start=(c == 0), stop=(c == DC - 1))
                    nc.scalar.activation(
                        Vaug[:, tcn, :, 0:64],
                        ps[:].rearrange("p (h d) -> p h d", d=64), AF.Identity)

            # ---- attention (ln1T freed)
            with ExitStack() as asc:
                pp = asc.enter_context(tc.tile_pool(name="pp", bufs=2))
                sps = asc.enter_context(
                    tc.tile_pool(name="sps", bufs=2, space="PSUM"))
                ops = asc.enter_context(
                    tc.tile_pool(name="ops", bufs=4, space="PSUM"))
                for h in range(H):
                    hk, hp = h // 2, (h % 2) * 64
                    KT_h = KT[hp:hp + 64, hk, :]
                    QT_h = QT[hp:hp + 64, hk, :]
                    P_t = [None] * NKB
                    for kb in range(NKB):
                        s0 = kb % NS
                        tail = TOWN - 128 * s0
                        st = sps.tile([128, TOWN], F32, tag="st")
                        for n0, w in _chunks(tail):
                            nc.tensor.matmul(
                                st[:, bass.ds(n0, w)],
                                KT_h[:, kb * 128:(kb + 1) * 128],
                                QT_h[:, bass.ds(TOWN - tail + n0, w)],
                                start=True, stop=True)
                        P_t[kb] = pp.tile([128, tail], BF16, tag=f"pt{kb}", name=f"pt{kb}")
                        nc.scalar.activation(P_t[kb][:], st[:, 0:tail],
                                             AF.Exp, scale=ISC)
                        m = tri_t if kb < NS else flag_t
                        nc.vector.tensor_mul(P_t[kb][:, 0:128],
                                             P_t[kb][:, 0:128], m[:])
                    for s in range(NS):
                        o = ops.tile([128, 65], F32, tag="o")
                        kbs = list(range(0, s + 1)) + list(range(NS, NS + s + 1))
                        for j, kb in enumerate(kbs):
                            c0 = (s - kb % NS) * 128
                            nc.tensor.matmul(o[:], P_t[kb][:, c0:c0 + 128],
                                             Vaug[:, kb, h, :],
                                             start=(j == 0), stop=(j == len(kbs) - 1))
                        rcp = pp.tile([128, 1], F32, tag="rcp")
                        nc.vector.reciprocal(rcp[:], o[:, 64:65])
                        nc.vector.scalar_tensor_tensor(
                            attn[:, s, h * 64:h * 64 + 64], o[:, 0:64], rcp[:],
                            vb_b[:, h * 64:h * 64 + 64], op0=ALU.mult, op1=ALU.add)

        # ---------------- proj + residual (QKV pools freed) ----------------
        xpool = top.enter_context(tc.tile_pool(name="xpool", bufs=1))
        xpb = xpool.tile([128, NS, D], F32, tag="xpb")  # x_own + proj_b; later out
        x2 = xpool.tile([128, NS, D], F32, tag="x2")
        with ExitStack() as s3:
            tp = s3.enter_context(tc.tile_pool(name="tp", bufs=1))
            tmp3 = s3.enter_context(tc.tile_pool(name="tmp3", bufs=2))
            tps = s3.enter_context(tc.tile_pool(name="tps", bufs=4, space="PSUM"))
            pps = s3.enter_context(tc.tile_pool(name="pps", bufs=2, space="PSUM"))
            for s in range(NS):
                xo = tmp3.tile([128, D], F32, tag="xo")
                nc.sync.dma_start(xo[:], t["x_own"][s * 128:(s + 1) * 128, :])
                nc.vector.tensor_add(xpb[:, s, :], xo[:], projb_b[:])
            attnT = tp.tile([128, DC, TOWN], BF16, tag="attnT")
            for s in range(NS):
                for c in range(DC):
                    pt = tps.tile([128, 128], BF16, tag="pt")
                    nc.tensor.transpose(pt[:], attn[:, s, c * 128:(c + 1) * 128],
                                        ident[:])
                    nc.scalar.activation(attnT[:, c, s * 128:(s + 1) * 128], pt[:],
                                         AF.Identity)
            wproj_t = tp.tile([128, DC, D], BF16, tag="wproj_t")
            for c in range(DC):
                nc.sync.dma_start(wproj_t[:, c, :],
                                  t["wproj"][c * 128:(c + 1) * 128, :])
            for s in range(NS):
                ps = pps.tile([128, D], F32, tag="proj_ps")
                for c in range(DC):
                    for n0, w in _chunks(D):
                        sl = bass.ds(n0, w)
                        nc.tensor.matmul(ps[:, sl],
                                         attnT[:, c, s * 128:(s + 1) * 128],
                                         wproj_t[:, c, :][:, sl],
                                         start=(c == 0), stop=(c == DC - 1))
                nc.vector.tensor_add(x2[:, s, :], ps[:], xpb[:, s, :])

        # ---------------- LN2 + fc1/gelu ----------------
        g1q = top.enter_context(tc.tile_pool(name="g1q", bufs=1))
        g1T = g1q.tile([128, HC, TOWN], BF16, tag="g1T")
        with ExitStack() as s4:
            l2 = s4.enter_context(tc.tile_pool(name="l2", bufs=2))
            l2q = s4.enter_context(tc.tile_pool(name="l2q", bufs=1))
            l2ps = s4.enter_context(tc.tile_pool(name="l2ps", bufs=2, space="PSUM"))
            ln2T = l2q.tile([128, DC, TOWN], BF16, tag="ln2T")
            for s in range(NS):
                ngr = max(1, D // 512)
                sg = l2.tile([128, ngr, 6], F32, tag="stats")
                for g in range(ngr):
                    nc.vector.bn_stats(sg[:, g, :],
                                       x2[:, s, g * (D // ngr):(g + 1) * (D // ngr)])
                mv = l2.tile([128, 2], F32, tag="mv")
                nc.vector.bn_aggr(mv[:], sg[:])
                sd = l2.tile([128, 1], F32, tag="sd2")
                nc.scalar.activation(sd[:], mv[:, 1:2], AF.Sqrt, bias=eps_t[:])
                rstd = l2.tile([128, 1], F32, tag="rstd2")
                nc.vector.reciprocal(rstd[:], sd[:])
                nmr = l2.tile([128, 1], F32, tag="nmr2")
                nc.vector.scalar_tensor_tensor(
                    nmr[:], mv[:, 0:1], -1.0, rstd[:], op0=ALU.mult, op1=ALU.mult)
                ln2n = l2.tile([128, D], BF16, tag="ln2n")
                nc.scalar.activation(ln2n[:], x2[:, s, :], AF.Identity,
                                     scale=rstd[:], bias=nmr[:])
                for c in range(DC):
                    pt = l2ps.tile([128, 128], BF16, tag="pt2")
                    nc.tensor.transpose(pt[:], ln2n[:, c * 128:(c + 1) * 128],
                                        ident[:])
                    nc.scalar.activation(ln2T[:, c, s * 128:(s + 1) * 128], pt[:],
                                         AF.Identity, scale=ln2g_t[:, c:c + 1],
                                         bias=ln2b_t[:, c:c + 1])
                nc.vector.tensor_add(x2[:, s, :], x2[:, s, :], fc2b_b[:])  # x2+fc2_b
            mw = s4.enter_context(tc.tile_pool(name="mw", bufs=3))
            mps = s4.enter_context(tc.tile_pool(name="mps", bufs=3, space="PSUM"))
            for hc in range(HC):
                w1 = mw.tile([128, DC, 128], BF16, tag="w1")
                nc.sync.dma_start(w1[:], t["wfc1t"][hc])
                ps = mps.tile([128, TOWN], F32, tag="h1ps")
                for c in range(DC):
                    for n0, w in _chunks(TOWN):
                        sl = bass.ds(n0, w)
                        nc.tensor.matmul(ps[:, sl], w1[:, c, :],
                                         ln2T[:, c, :][:, sl],
                                         start=(c == 0), stop=(c == DC - 1))
                nc.scalar.activation(g1T[:, hc, :], ps[:], AF.Gelu,
                                     bias=fc1b_t[:, hc:hc + 1])
        # ---------------- fc2 + residual + store ----------------
        with ExitStack() as s5:
            f2 = s5.enter_context(tc.tile_pool(name="f2", bufs=3))
            f2ps = s5.enter_context(tc.tile_pool(name="f2ps", bufs=1, space="PSUM"))
            half = D // 2
            for dg in range(2):
                dsl = bass.ts(dg, half)
                pss = [f2ps.tile([128, half], F32, tag=f"f2ps{s}", name=f"f2ps{s}")
                       for s in range(NS)]
                for hc in range(HC):
                    w2 = f2.tile([128, half], BF16, tag="w2")
                    nc.sync.dma_start(
                        w2[:],
                        t["wfc2"][hc * 128:(hc + 1) * 128, dg * half:(dg + 1) * half])
                    for s in range(NS):
                        nc.tensor.matmul(pss[s][:],
                                         g1T[:, hc, s * 128:(s + 1) * 128],
                                         w2[:], start=(hc == 0), stop=(hc == HC - 1))
                for s in range(NS):
                    nc.vector.tensor_add(xpb[:, s, dsl], pss[s][:], x2[:, s, dsl])
                    nc.sync.dma_start(t["out"][s * 128:(s + 1) * 128, dsl],
                                      xpb[:, s, dsl])


# ---------------------------------------------------------------------------
# Host side
# ---------------------------------------------------------------------------

def host_inputs(cfg, B, x, ln1_g, ln1_b, qkv_w, qkv_b, proj_w, proj_b,
                ln2_g, ln2_b, fc1_w, fc1_b, fc2_w, fc2_b):
    """Build per-core input maps (2 cores per batch element)."""
    T, D, HID = cfg["T"], cfg["D"], cfg["HID"]
    DC, HC = D // 128, HID // 128
    TOWN, NS = T // 2, T // 256
    bf = ml_dtypes.bfloat16

    def pp(vec, nch):
        return np.ascontiguousarray(
            np.asarray(vec, np.float32).reshape(nch, 128).T)

    qkv_b = np.asarray(qkv_b, np.float32)
    qb, kb, vb = qkv_b[0:D], qkv_b[D:2 * D], qkv_b[2 * D:3 * D]
    wfc1t = np.ascontiguousarray(
        np.asarray(fc1_w, np.float32).reshape(DC, 128, HC, 128)
        .transpose(2, 1, 0, 3)).astype(bf)
    shared = dict(
        wq=np.asarray(qkv_w[:, 0:D], np.float32).astype(bf),
        wk=np.asarray(qkv_w[:, D:2 * D], np.float32).astype(bf),
        wv=np.asarray(qkv_w[:, 2 * D:3 * D], np.float32).astype(bf),
        qb_pp=pp(qb, DC), kb_pp=pp(kb, DC),
        vb_row=np.ascontiguousarray(vb.reshape(1, D)),
        ln1g_pp=pp(ln1_g, DC), ln1b_pp=pp(ln1_b, DC),
        ln2g_pp=pp(ln2_g, DC), ln2b_pp=pp(ln2_b, DC),
        wproj=np.asarray(proj_w, np.float32).astype(bf),
        projb_row=np.asarray(proj_b, np.float32).reshape(1, D),
        wfc1t=wfc1t,
        fc1b_pp=pp(fc1_b, HC),
        wfc2=np.asarray(fc2_w, np.float32).astype(bf),
        fc2b_row=np.asarray(fc2_b, np.float32).reshape(1, D),
    )
    x = np.asarray(x, np.float32)
    qi = np.arange(128)[None, :]
    k2 = np.arange(128)[:, None]
    tri = (k2 <= qi).astype(bf)
    in_maps, owns = [], []
    for b in range(B):
        for pr in range(2):
            own = np.concatenate(
                [np.arange(256 * s + 128 * pr, 256 * s + 128 * pr + 128)
                 for s in range(NS)])
            other = np.concatenate(
                [np.arange(256 * s + 128 * (1 - pr), 256 * s + 128 * (1 - pr) + 128)
                 for s in range(NS)])
            perm = np.concatenate([own, other])
            m = dict(shared)
            m["xT"] = np.ascontiguousarray(x[b][perm, :].T)
            m["x_own"] = np.ascontiguousarray(x[b][own, :])
            m["tri"] = tri
            m["flag"] = np.full((128, 128), float(pr), bf)
            in_maps.append(m)
            owns.append((b, own))
    return in_maps, owns


_PROGRAM_CACHE = {}


def get_program(cfg):
    key = tuple(sorted(cfg.items()))
    if key not in _PROGRAM_CACHE:
        _PROGRAM_CACHE[key] = build_program(cfg)
    return _PROGRAM_CACHE[key]


def kernel(x, ln1_g, ln1_b, qkv_w, qkv_b, proj_w, proj_b,
           ln2_g, ln2_b, fc1_w, fc1_b, fc2_w, fc2_b, _trace=False):
    from concourse.bass_utils import run_bass_kernel_spmd
    cfg = FULL_CFG
    nc = get_program(cfg)
    x = np.asarray(x, np.float32)
    B = x.shape[0]
    in_maps, owns = host_inputs(
        cfg, B, x, ln1_g, ln1_b, qkv_w, qkv_b, proj_w, proj_b,
        ln2_g, ln2_b, fc1_w, fc1_b, fc2_w, fc2_b)
    res = run_bass_kernel_spmd(nc, in_maps, list(range(len(in_maps))))
    T, D = cfg["T"], cfg["D"]
    outp = np.empty((B, T, D), np.float32)
    for (b, own), r in zip(owns, res.results):
        outp[b, own, :] = r["out"]
    if _trace:
        kernel.last_result = res
    return outp


# revision 8
# speedup vs baseline: 3.3129x; 3.3129x over previous
"""TRN2 Bass kernel for nn_DecoderBlock (B=4, T=2048, D=1024, H=16, HID=4096).

Sharding: 8 cores = batch(4) x causal-balanced T-split(2).  Core (b, p) owns
token blocks {256*s + 128*p : +128, s=0..7} of batch b (interleaved 128-row
blocks; per-core causal work is exactly balanced).  Each core redundantly
computes LN1 + K/V over the full sequence of its batch element, so there is
no cross-core communication at all; attention Q/scores/output, proj, LN2 and
the MLP are computed only for the core's own 1024 rows.  Host reassembles the
8 [1024, 1024] outputs.

Token columns are permuted per core to [own | other] so the SPMD program is
identical across cores; the only data-dependent piece of causal masking is a
[128,128] triangular constant and a [128,128] all-0/all-1 flag tile.

Layouts (per core):
  xT     [D, T]    fp32   x[b].T (token-permuted)
  ln1T   [D, T]    bf16   layernorm1, transposed; stats via PE ones-matmul
  QT     [D, TOWN] bf16   Q^T   KT [D, T] bf16   K^T   (weights-stationary)
  V_aug  [128, T/128, H, 65] bf16  V natural + ones column (softmax denom)
  P^T    [128, tail] bf16  exp(scores^T), per k-block, masked
  O      [128, 65] psum    P^T-stationary AV matmul -> per-q denom in col 64
  attn   [TOWN, D] bf16 natural -> PE-transposed for proj
  x2/out fp32 residual stream end-to-end
"""
import numpy as np
import ml_dtypes
from contextlib import ExitStack

import concourse.bass as bass
import concourse.bacc as bacc
import concourse.mybir as mybir
import concourse.tile as tile
from concourse.masks import make_identity

def _chunks(total, step=512):
    return [(i, min(step, total - i)) for i in range(0, total, step)]


F32 = mybir.dt.float32
BF16 = mybir.dt.bfloat16
AF = mybir.ActivationFunctionType
ALU = mybir.AluOpType

FULL_CFG = dict(T=2048, D=1024, HID=4096)
EPS = 1e-5


def build_program(cfg, reps=1):
    T, D, HID = cfg["T"], cfg["D"], cfg["HID"]
    HD = 64
    p = dict(
        T=T, D=D, HID=HID,
        H=D // HD, DC=D // 128, HC=HID // 128,
        TOWN=T // 2, NS=T // 256, NKB=T // 128, TC=T // 128,
        ISC=float(1.0 / np.sqrt(np.float32(HD))),
    )
    nc = bacc.Bacc("TRN2", target_bir_lowering=False, debug=False)

    def din(name, shape, dtype=F32):
        return nc.dram_tensor(name, shape, dtype, kind="ExternalInput").ap()

    DC, HC = p["DC"], p["HC"]
    t = dict(
        xT=din("xT", [D, T]),
        x_own=din("x_own", [p["TOWN"], D]),
        tri=din("tri", [128, 128], BF16),
        flag=din("flag", [128, 128], BF16),
        wq=din("wq", [D, D], BF16),
        wk=din("wk", [D, D], BF16),
        wv=din("wv", [D, D], BF16),
        qb_pp=din("qb_pp", [128, DC]),
        kb_pp=din("kb_pp", [128, DC]),
        vb_row=din("vb_row", [1, D]),
        ln1g_pp=din("ln1g_pp", [128, DC]),
        ln1b_pp=din("ln1b_pp", [128, DC]),
        ln2g_pp=din("ln2g_pp", [128, DC]),
        ln2b_pp=din("ln2b_pp", [128, DC]),
        wproj=din("wproj", [D, D], BF16),
        projb_row=din("projb_row", [1, D]),
        wfc1t=din("wfc1t", [HC, 128, DC, 128], BF16),
        fc1b_pp=din("fc1b_pp", [128, HC]),
        wfc2=din("wfc2", [HID, D], BF16),
        fc2b_row=din("fc2b_row", [1, D]),
        out=nc.dram_tensor("out", [p["TOWN"], D], F32, kind="ExternalOutput").ap(),
    )
    with tile.TileContext(nc) as tc:
        _body(tc, p, t, reps=reps)
    nc.compile()
    return nc


def _body(tc, p, t, reps=1):
    nc = tc.nc
    T, D, HID = p["T"], p["D"], p["HID"]
    H, DC, HC = p["H"], p["DC"], p["HC"]
    TOWN, NS, NKB, TC, ISC = p["TOWN"], p["NS"], p["NKB"], p["TC"], p["ISC"]

    with ExitStack() as top:
        consts = top.enter_context(tc.tile_pool(name="consts", bufs=1))
        ident = consts.tile([128, 128], BF16, tag="ident")
        make_identity(nc, ident[:])
        tri_t = consts.tile([128, 128], BF16, tag="tri")
        nc.sync.dma_start(tri_t[:], t["tri"][:])
        flag_t = consts.tile([128, 128], BF16, tag="flag")
        nc.sync.dma_start(flag_t[:], t["flag"][:])
        ppvec = consts.tile([128, DC, 6], F32, tag="ppvec")
        for i, nm in enumerate(["qb_pp", "kb_pp", "ln1g_pp", "ln1b_pp",
                                "ln2g_pp", "ln2b_pp"]):
            nc.sync.dma_start(ppvec[:, :, i], t[nm][:])
        qb_t, kb_t = ppvec[:, :, 0], ppvec[:, :, 1]
        ln1g_t, ln1b_t = ppvec[:, :, 2], ppvec[:, :, 3]
        ln2g_t, ln2b_t = ppvec[:, :, 4], ppvec[:, :, 5]
        fc1b_t = consts.tile([128, HC], F32, tag="fc1b")
        nc.sync.dma_start(fc1b_t[:], t["fc1b_pp"][:])
        vb_b = consts.tile([128, D], F32, tag="vb_b")
        nc.sync.dma_start(vb_b[:], t["vb_row"][:].broadcast_to([128, D]))
        projb_b = consts.tile([128, D], F32, tag="projb_b")
        nc.sync.dma_start(projb_b[:], t["projb_row"][:].broadcast_to([128, D]))
        fc2b_b = consts.tile([128, D], F32, tag="fc2b_b")
        nc.sync.dma_start(fc2b_b[:], t["fc2b_row"][:].broadcast_to([128, D]))
        ones_f = consts.tile([128, 128], F32, tag="ones_f")
        nc.vector.memset(ones_f[:], 1.0)
        eps_t = consts.tile([128, 1], F32, tag="eps")
        nc.vector.memset(eps_t[:], EPS)

        for _rep in range(reps):
         with ExitStack() as top2:
          if True:
            attnp = top2.enter_context(tc.tile_pool(name="attnp", bufs=1))
            attn = attnp.tile([128, NS, D], BF16, tag="attn", name="attn")
            _one_pass(tc, p, t, locals())


def _one_pass(tc, p, t, env):
    nc = tc.nc
    T, D, HID = p["T"], p["D"], p["HID"]
    H, DC, HC = p["H"], p["DC"], p["HC"]
    TOWN, NS, NKB, TC, ISC = p["TOWN"], p["NS"], p["NKB"], p["TC"], p["ISC"]
    top2 = env["top2"]
    attn = env["attn"]
    ident, tri_t, flag_t = env["ident"], env["tri_t"], env["flag_t"]
    qb_t, kb_t = env["qb_t"], env["kb_t"]
    ln1g_t, ln1b_t = env["ln1g_t"], env["ln1b_t"]
    ln2g_t, ln2b_t = env["ln2g_t"], env["ln2b_t"]
    fc1b_t, vb_b, projb_b, fc2b_b = (env["fc1b_t"], env["vb_b"],
                                     env["projb_b"], env["fc2b_b"])
    ones_f, eps_t = env["ones_f"], env["eps_t"]
    if True:
        with ExitStack() as qsc:  # LN1 + QKV + attention
            qkvp = qsc.enter_context(tc.tile_pool(name="qkvp", bufs=1))
            QT = qkvp.tile([128, DC, TOWN], BF16, tag="QT")
            KT = qkvp.tile([128, DC, T], BF16, tag="KT")
            Vaug = qkvp.tile([128, TC, H, 65], BF16, tag="Vaug")
            nc.gpsimd.memset(Vaug[:, :, :, 64:65], 1.0)

            with ExitStack() as lsc:  # LN1 + QKV
                lnq = lsc.enter_context(tc.tile_pool(name="lnq", bufs=1))
                ln1T = lnq.tile([128, DC, T], BF16, tag="ln1T")
                with ExitStack() as l1:
                    lnp = l1.enter_context(tc.tile_pool(name="lnp", bufs=2))
                    lns = l1.enter_context(tc.tile_pool(name="lns", bufs=1))
                    lps = l1.enter_context(
                        tc.tile_pool(name="lnps", bufs=1, space="PSUM"))
                    S1b = lps.tile([128, T], F32, tag="S1b")
                    S2b = lps.tile([128, T], F32, tag="S2b")
                    for c in range(DC):
                        xc = lnp.tile([128, T], F32, tag="xc")
                        nc.sync.dma_start(xc[:], t["xT"][c * 128:(c + 1) * 128, :])
                        sq = lnp.tile([128, T], F32, tag="scr")
                        nc.scalar.activation(sq[:], xc[:], AF.Square)
                        for n0, w in _chunks(T):
                            sl = bass.ds(n0, w)
                            nc.tensor.matmul(S1b[:, sl], ones_f[:], xc[:, sl],
                                             start=(c == 0), stop=(c == DC - 1))
                            nc.tensor.matmul(S2b[:, sl], ones_f[:], sq[:, sl],
                                             start=(c == 0), stop=(c == DC - 1))
                    # stats in broadcast layout [128, T]; A=mu, B=ex2->var->rstd, C=scratch->nmr
                    A = lns.tile([128, T], F32, tag="A")
                    B = lns.tile([128, T], F32, tag="B")
                    C = lns.tile([128, T], F32, tag="C")
                    nc.vector.tensor_scalar_mul(A[:], S1b[:], 1.0 / D)
                    nc.vector.tensor_scalar_mul(B[:], S2b[:], 1.0 / D)
                    nc.vector.scalar_tensor_tensor(
                        C[:], A[:], -1.0, A[:], op0=ALU.mult, op1=ALU.mult)
                    nc.vector.tensor_add(B[:], B[:], C[:])          # var
                    nc.scalar.activation(C[:], B[:], AF.Sqrt, bias=eps_t[:])
                    nc.vector.reciprocal(B[:], C[:])                # rstd
                    nc.vector.scalar_tensor_tensor(
                        C[:], A[:], -1.0, B[:], op0=ALU.mult, op1=ALU.mult)  # -mu*rstd
                    for c in range(DC):
                        xc = lnp.tile([128, T], F32, tag="xc")
                        nc.sync.dma_start(xc[:], t["xT"][c * 128:(c + 1) * 128, :])
                        t1 = lnp.tile([128, T], F32, tag="scr")
                        nc.vector.tensor_mul(t1[:], xc[:], B[:])
                        nc.vector.tensor_add(t1[:], t1[:], C[:])
                        nc.scalar.activation(
                            ln1T[:, c, :], t1[:], AF.Identity,
                            scale=ln1g_t[:, c:c + 1], bias=ln1b_t[:, c:c + 1])

                # ---- QKV (weights streamed per D-chunk, psum per out-chunk)
                wst = lsc.enter_context(tc.tile_pool(name="wst", bufs=2))
                qps = lsc.enter_context(
                    tc.tile_pool(name="qps", bufs=2, space="PSUM"))
                w_t = [None] * DC
                for c in range(DC):
                    w_t[c] = wst.tile([128, D], BF16, tag=f"w{c}", name=f"w{c}")
                    nc.sync.dma_start(w_t[c][:], t["wq"][c * 128:(c + 1) * 128, :])
                for qc in range(DC):
                    ps = qps.tile([128, TOWN], F32, tag="qkv_ps")
                    for c in range(DC):
                        for n0, w in _chunks(TOWN):
                            sl = bass.ds(n0, w)
                            nc.tensor.matmul(ps[:, sl],
                                             w_t[c][:, qc * 128:(qc + 1) * 128],
                                             ln1T[:, c, 0:TOWN][:, sl],
                                             start=(c == 0), stop=(c == DC - 1))
                    nc.scalar.activation(QT[:, qc, :], ps[:], AF.Identity,
                                         bias=qb_t[:, qc:qc + 1])
                for c in range(DC):
                    w_t[c] = wst.tile([128, D], BF16, tag=f"w{c}", name=f"w{c}")
                    nc.sync.dma_start(w_t[c][:], t["wk"][c * 128:(c + 1) * 128, :])
                for kc in range(DC):
                    for th in range(T // TOWN):
                        ps = qps.tile([128, TOWN], F32, tag="qkv_ps")
                        base = th * TOWN
                        for c in range(DC):
                            for n0, w in _chunks(TOWN):
                                nc.tensor.matmul(
                                    ps[:, bass.ds(n0, w)],
                                    w_t[c][:, kc * 128:(kc + 1) * 128],
                                    ln1T[:, c, :][:, bass.ds(base + n0, w)],
                                    start=(c == 0), stop=(c == DC - 1))
                        nc.scalar.activation(KT[:, kc, base:base + TOWN], ps[:],
                                             AF.Identity, bias=kb_t[:, kc:kc + 1])
                for c in range(DC):
                    w_t[c] = wst.tile([128, D], BF16, tag=f"w{c}", name=f"w{c}")
                    nc.sync.dma_start(w_t[c][:], t["wv"][c * 128:(c + 1) * 128, :])
                for tcn in range(TC):
                    ps = qps.tile([128, D], F32, tag="qkv_ps")
                    for c in range(DC):
                        for n0, w in _chunks(D):
                            sl = bass.ds(n0, w)
                            nc.tensor.matmul(ps[:, sl],
                                             ln1T[:, c, tcn * 128:(tcn + 1) * 128],
                                             w_t[c][:, sl],
                                             start=(c == 0), stop=(c == DC - 1))
                    nc.scalar.activation(
                        Vaug[:, tcn, :, 0:64],
                        ps[:].rearrange("p (h d) -> p h d", d=64), AF.Identity)

            # ---- attention (ln1T freed)
            with ExitStack() as asc:
                pp = asc.enter_context(tc.tile_pool(name="pp", bufs=2))
                sps = asc.enter_context(
                    tc.tile_pool(name="sps", bufs=2, space="PSUM"))
                ops = asc.enter_context(
                    tc.tile_pool(name="ops", bufs=4, space="PSUM"))
                for h in range(H):
                    hk, hp = h // 2, (h % 2) * 64
                    KT_h = KT[hp:hp + 64, hk, :]
                    QT_h = QT[hp:hp + 64, hk, :]
                    P_t = [None] * NKB
                    for kb in range(NKB):
                        s0 = kb % NS
                        tail = TOWN - 128 * s0
                        st = sps.tile([128, TOWN], F32, tag="st")
                        for n0, w in _chunks(tail):
                            nc.tensor.matmul(
                                st[:, bass.ds(n0, w)],
                                KT_h[:, kb * 128:(kb + 1) * 128],
                                QT_h[:, bass.ds(TOWN - tail + n0, w)],
                                start=True, stop=True)
                        P_t[kb] = pp.tile([128, tail], BF16, tag=f"pt{kb}", name=f"pt{kb}")
                        nc.scalar.activation(P_t[kb][:], st[:, 0:tail],
                                             AF.Exp, scale=ISC)
                        m = tri_t if kb < NS else flag_t
                        nc.vector.tensor_mul(P_t[kb][:, 0:128],
                                             P_t[kb][:, 0:128], m[:])
                    for s in range(NS):
                        o = ops.tile([128, 65], F32, tag="o")
                        kbs = list(range(0, s + 1)) + list(range(NS, NS + s + 1))
                        for j, kb in enumerate(kbs):
                            c0 = (s - kb % NS) * 128
                            nc.tensor.matmul(o[:], P_t[kb][:, c0:c0 + 128],
                                             Vaug[:, kb, h, :],
                                             start=(j == 0), stop=(j == len(kbs) - 1))
                        rcp = pp.tile([128, 1], F32, tag="rcp")
                        nc.vector.reciprocal(rcp[:], o[:, 64:65])
                        nc.vector.scalar_tensor_tensor(
                            attn[:, s, h * 64:h * 64 + 64], o[:, 0:64], rcp[:],
                            vb_b[:, h * 64:h * 64 + 64], op0=ALU.mult, op1=ALU.add)

        # ---------------- proj + residual (QKV pools freed) ----------------
        xpool = top2.enter_context(tc.tile_pool(name="xpool", bufs=1))
        xpb = xpool.tile([128, NS, D], F32, tag="xpb")  # x_own + proj_b; later out
        x2 = xpool.tile([128, NS, D], F32, tag="x2")
        with ExitStack() as s3:
            tp = s3.enter_context(tc.tile_pool(name="tp", bufs=1))
            tmp3 = s3.enter_context(tc.tile_pool(name="tmp3", bufs=2))
            tps = s3.enter_context(tc.tile_pool(name="tps", bufs=4, space="PSUM"))
            pps = s3.enter_context(tc.tile_pool(name="pps", bufs=2, space="PSUM"))
            for s in range(NS):
                xo = tmp3.tile([128, D], F32, tag="xo")
                nc.sync.dma_start(xo[:], t["x_own"][s * 128:(s + 1) * 128, :])
                nc.vector.tensor_add(xpb[:, s, :], xo[:], projb_b[:])
            attnT = tp.tile([128, DC, TOWN], BF16, tag="attnT")
            for s in range(NS):
                for c in range(DC):
                    pt = tps.tile([128, 128], BF16, tag="pt")
                    nc.tensor.transpose(pt[:], attn[:, s, c * 128:(c + 1) * 128],
                                        ident[:])
                    nc.scalar.activation(attnT[:, c, s * 128:(s + 1) * 128], pt[:],
                                         AF.Identity)
            wproj_t = tp.tile([128, DC, D], BF16, tag="wproj_t")
            for c in range(DC):
                nc.sync.dma_start(wproj_t[:, c, :],
                                  t["wproj"][c * 128:(c + 1) * 128, :])
            for s in range(NS):
                ps = pps.tile([128, D], F32, tag="proj_ps")
                for c in range(DC):
                    for n0, w in _chunks(D):
                        sl = bass.ds(n0, w)
                        nc.tensor.matmul(ps[:, sl],
                                         attnT[:, c, s * 128:(s + 1) * 128],
                                         wproj_t[:, c, :][:, sl],
                                         start=(c == 0), stop=(c == DC - 1))
                nc.vector.tensor_add(x2[:, s, :], ps[:], xpb[:, s, :])

        # ---------------- LN2 + fc1/gelu ----------------
        g1q = top2.enter_context(tc.tile_pool(name="g1q", bufs=1))
        g1T = g1q.tile([128, HC, TOWN], BF16, tag="g1T")
        with ExitStack() as s4:
            l2 = s4.enter_context(tc.tile_pool(name="l2", bufs=2))
            l2q = s4.enter_context(tc.tile_pool(name="l2q", bufs=1))
            l2ps = s4.enter_context(tc.tile_pool(name="l2ps", bufs=2, space="PSUM"))
            ln2T = l2q.tile([128, DC, TOWN], BF16, tag="ln2T")
            for s in range(NS):
                ngr = max(1, D // 512)
                sg = l2.tile([128, ngr, 6], F32, tag="stats")
                for g in range(ngr):
                    nc.vector.bn_stats(sg[:, g, :],
                                       x2[:, s, g * (D // ngr):(g + 1) * (D // ngr)])
                mv = l2.tile([128, 2], F32, tag="mv")
                nc.vector.bn_aggr(mv[:], sg[:])
                sd = l2.tile([128, 1], F32, tag="sd2")
                nc.scalar.activation(sd[:], mv[:, 1:2], AF.Sqrt, bias=eps_t[:])
                rstd = l2.tile([128, 1], F32, tag="rstd2")
                nc.vector.reciprocal(rstd[:], sd[:])
                nmr = l2.tile([128, 1], F32, tag="nmr2")
                nc.vector.scalar_tensor_tensor(
                    nmr[:], mv[:, 0:1], -1.0, rstd[:], op0=ALU.mult, op1=ALU.mult)
                ln2n = l2.tile([128, D], BF16, tag="ln2n")
                nc.scalar.activation(ln2n[:], x2[:, s, :], AF.Identity,
                                     scale=rstd[:], bias=nmr[:])
                for c in range(DC):
                    pt = l2ps.tile([128, 128], BF16, tag="pt2")
                    nc.tensor.transpose(pt[:], ln2n[:, c * 128:(c + 1) * 128],
                                        ident[:])
                    nc.scalar.activation(ln2T[:, c, s * 128:(s + 1) * 128], pt[:],
                                         AF.Identity, scale=ln2g_t[:, c:c + 1],
                                         bias=ln2b_t[:, c:c + 1])
                nc.vector.tensor_add(x2[:, s, :], x2[:, s, :], fc2b_b[:])  # x2+fc2_b
            mw = s4.enter_context(tc.tile_pool(name="mw", bufs=3))
            mps = s4.enter_context(tc.tile_pool(name="mps", bufs=3, space="PSUM"))
            for hc in range(HC):
                w1 = mw.tile([128, DC, 128], BF16, tag="w1")
                nc.sync.dma_start(w1[:], t["wfc1t"][hc])
                ps = mps.tile([128, TOWN], F32, tag="h1ps")
                for c in range(DC):
                    for n0, w in _chunks(TOWN):
                        sl = bass.ds(n0, w)
                        nc.tensor.matmul(ps[:, sl], w1[:, c, :],
                                         ln2T[:, c, :][:, sl],
                                         start=(c == 0), stop=(c == DC - 1))
                nc.scalar.activation(g1T[:, hc, :], ps[:], AF.Gelu,
                                     bias=fc1b_t[:, hc:hc + 1])
        # ---------------- fc2 + residual + store ----------------
        with ExitStack() as s5:
            f2 = s5.enter_context(tc.tile_pool(name="f2", bufs=3))
            f2ps = s5.enter_context(tc.tile_pool(name="f2ps", bufs=1, space="PSUM"))
            half = D // 2
            for dg in range(2):
                dsl = bass.ts(dg, half)
                pss = [f2ps.tile([128, half], F32, tag=f"f2ps{s}", name=f"f2ps{s}")
                       for s in range(NS)]
                for hc in range(HC):
                    w2 = f2.tile([128, half], BF16, tag="w2")
                    nc.sync.dma_start(
                        w2[:],
                        t["wfc2"][hc * 128:(hc + 1) * 128, dg * half:(dg + 1) * half])
                    for s in range(NS):
                        nc.tensor.matmul(pss[s][:],
                                         g1T[:, hc, s * 128:(s + 1) * 128],
                                         w2[:], start=(hc == 0), stop=(hc == HC - 1))
                for s in range(NS):
                    nc.vector.tensor_add(xpb[:, s, dsl], pss[s][:], x2[:, s, dsl])
                    nc.sync.dma_start(t["out"][s * 128:(s + 1) * 128, dsl],
                                      xpb[:, s, dsl])


# ---------------------------------------------------------------------------
# Host side
# ---------------------------------------------------------------------------

def host_inputs(cfg, B, x, ln1_g, ln1_b, qkv_w, qkv_b, proj_w, proj_b,
                ln2_g, ln2_b, fc1_w, fc1_b, fc2_w, fc2_b):
    """Build per-core input maps (2 cores per batch element)."""
    T, D, HID = cfg["T"], cfg["D"], cfg["HID"]
    DC, HC = D // 128, HID // 128
    TOWN, NS = T // 2, T // 256
    bf = ml_dtypes.bfloat16

    def pp(vec, nch):
        return np.ascontiguousarray(
            np.asarray(vec, np.float32).reshape(nch, 128).T)

    qkv_b = np.asarray(qkv_b, np.float32)
    qb, kb, vb = qkv_b[0:D], qkv_b[D:2 * D], qkv_b[2 * D:3 * D]
    wfc1t = np.ascontiguousarray(
        np.asarray(fc1_w, np.float32).reshape(DC, 128, HC, 128)
        .transpose(2, 1, 0, 3)).astype(bf)
    shared = dict(
        wq=np.asarray(qkv_w[:, 0:D], np.float32).astype(bf),
        wk=np.asarray(qkv_w[:, D:2 * D], np.float32).astype(bf),
        wv=np.asarray(qkv_w[:, 2 * D:3 * D], np.float32).astype(bf),
        qb_pp=pp(qb, DC), kb_pp=pp(kb, DC),
        vb_row=np.ascontiguousarray(vb.reshape(1, D)),
        ln1g_pp=pp(ln1_g, DC), ln1b_pp=pp(ln1_b, DC),
        ln2g_pp=pp(ln2_g, DC), ln2b_pp=pp(ln2_b, DC),
        wproj=np.asarray(proj_w, np.float32).astype(bf),
        projb_row=np.asarray(proj_b, np.float32).reshape(1, D),
        wfc1t=wfc1t,
        fc1b_pp=pp(fc1_b, HC),
        wfc2=np.asarray(fc2_w, np.float32).astype(bf),
        fc2b_row=np.asarray(fc2_b, np.float32).reshape(1, D),
    )
    x = np.asarray(x, np.float32)
    qi = np.arange(128)[None, :]
    k2 = np.arange(128)[:, None]
    tri = (k2 <= qi).astype(bf)
    in_maps, owns = [], []
    for b in range(B):
        for pr in range(2):
            own = np.concatenate(
                [np.arange(256 * s + 128 * pr, 256 * s + 128 * pr + 128)
                 for s in range(NS)])
            other = np.concatenate(
                [np.arange(256 * s + 128 * (1 - pr), 256 * s + 128 * (1 - pr) + 128)
                 for s in range(NS)])
            perm = np.concatenate([own, other])
            m = dict(shared)
            m["xT"] = np.ascontiguousarray(x[b][perm, :].T)
            m["x_own"] = np.ascontiguousarray(x[b][own, :])
            m["tri"] = tri
            m["flag"] = np.full((128, 128), float(pr), bf)
            in_maps.append(m)
            owns.append((b, own))
    return in_maps, owns


_PROGRAM_CACHE = {}


def get_program(cfg):
    key = tuple(sorted(cfg.items()))
    if key not in _PROGRAM_CACHE:
        _PROGRAM_CACHE[key] = build_program(cfg)
    return _PROGRAM_CACHE[key]


def kernel(x, ln1_g, ln1_b, qkv_w, qkv_b, proj_w, proj_b,
           ln2_g, ln2_b, fc1_w, fc1_b, fc2_w, fc2_b, _trace=False):
    from concourse.bass_utils import run_bass_kernel_spmd
    cfg = FULL_CFG
    nc = get_program(cfg)
    x = np.asarray(x, np.float32)
    B = x.shape[0]
    in_maps, owns = host_inputs(
        cfg, B, x, ln1_g, ln1_b, qkv_w, qkv_b, proj_w, proj_b,
        ln2_g, ln2_b, fc1_w, fc1_b, fc2_w, fc2_b)
    res = run_bass_kernel_spmd(nc, in_maps, list(range(len(in_maps))))
    T, D = cfg["T"], cfg["D"]
    outp = np.empty((B, T, D), np.float32)
    for (b, own), r in zip(owns, res.results):
        outp[b, own, :] = r["out"]
    if _trace:
        kernel.last_result = res
    return outp
